# revision 8
# baseline (speedup 1.0000x reference)
"""BitLinear forward (RMSNorm -> int8 activation quant -> ternary weight quant
-> matmul -> rescale) on 8 Trainium2 NeuronCores.

Sharding: data-parallel over rows. x (4,4096,1024) flattens to (16384,1024);
each core gets 2048 rows and the full weight (4096,1024). w_scale=mean|w| is
computed locally per core from a single pass over the full weight (the
per-shard approximation fails the tolerance; a collective AllReduce costs
~55us of latency, while the single-pass local sum is DMA-bandwidth-bound at
~50us anyway and needs no collective).

v2 schedule (single kernel, emission order == per-engine FIFO order):
 - scalar HWDGE ring: 32 w-strip loads (pass 1), then all output stores.
 - sync HWDGE ring: x tiles + re-loads of the strips not held in SBUF.
 - DVE: |w| abs-accum per strip (keeps ACT free for DMA triggers), x quant
   chains, ternarize clip ops, half the PSUM evacuations.
 - ACT: sqrt + RNE-scale ops of x quant, h1 ternarize cast, half the PSUM
   evacuations.
 - GPSIMD: partition all-reduce for w_scale, h0 ternarize cast, x ssq/xq.
 - PE: identity-matmul transposes + the 1024 main matmuls; strict queue
   order chosen so the PE never head-of-line blocks on not-yet-ready work
   (that blocking caused ~110us of PE idle + HAM re-throttle in v1).

Math notes:
 - x_q are exact integers in [-128,127] and w_t in {-1,0,1}; both are exact in
   bf16, so a bf16 matmul with fp32 PSUM accumulation reproduces the fp32
   reference einsum bit-for-bit (|sums| < 2^24).
 - round-half-to-even is done in fp32 via the magic constant 1.5*2^23.
 - ternary quantize sign(ws)*(|ws|>0.5) == RNE(clip(ws,-1,1)) exactly.
 - transposes to [k, r]/[k, n] layouts are identity matmuls (out = a.T @ I),
   batched 4 chunks per PSUM bank with one wide copy back to SBUF.
"""

import os

import numpy as np

import concourse.bass as bass
import concourse.mybir as mybir
import concourse.tile as tile
from concourse import bacc
from concourse.bass_utils import run_bass_kernel_spmd
from concourse.masks import make_identity
from concourse import bass_isa

F32 = mybir.dt.float32
BF16 = mybir.dt.bfloat16
ALU = mybir.AluOpType
AF = mybir.ActivationFunctionType

N_CORES = 8
R_FULL, K, N = 16384, 1024, 4096
R = R_FULL // N_CORES          # 2048 rows per core
RT = R // 128                  # 16 row tiles per core
KC = K // 128                  # 8 k-chunks
WS = N // 128                  # 32 weight strips (of 128 out-features)
NH = 2                         # n halves (2048 each)
S_HOLD = 8                     # strips kept resident between pass1 and burst

C_MAGIC = 12582912.0           # 1.5 * 2^23: fp32 round-to-nearest-even trick
Q_EPS = 1e-5
NORM_EPS = 1e-6


def build_nc(g_is_ones: bool):
    nc = bacc.Bacc("TRN2", target_bir_lowering=False)

    x_d = nc.dram_tensor("x", [R, K], F32, kind="ExternalInput")
    w_d = nc.dram_tensor("w", [N, K], F32, kind="ExternalInput")
    if not g_is_ones:
        g_d = nc.dram_tensor("g", [1, K], F32, kind="ExternalInput")
    out_d = nc.dram_tensor("out", [R, N], F32, kind="ExternalOutput")

    with tile.TileContext(nc) as tc:
        with (
            tc.tile_pool(name="persist", bufs=1) as persist,
            tc.tile_pool(name="xt", bufs=2) as xt_pool,
            tc.tile_pool(name="scr", bufs=2) as scr_pool,       # bf16 scratch
            tc.tile_pool(name="st", bufs=2) as st_pool,         # [128,1] stats
            tc.tile_pool(name="ux", bufs=1) as ux_pool,
            tc.tile_pool(name="xqp", bufs=2) as xq_pool,
            tc.tile_pool(name="xqT", bufs=14) as xqT_pool,
            tc.tile_pool(name="w1", bufs=3) as w1_pool,         # w pass1 + reloads
            tc.tile_pool(name="h1p", bufs=3) as h1_pool,        # h1 reloads
            tc.tile_pool(name="uv", bufs=2) as uv_pool,
            tc.tile_pool(name="wtn", bufs=2) as wtn_pool,
            tc.tile_pool(name="stg", bufs=2) as stage_pool,
            tc.tile_pool(name="csp", bufs=16) as cs_pool,
            tc.tile_pool(name="pmm", bufs=6, space="PSUM") as psum_mm,
            tc.tile_pool(name="ptp", bufs=2, space="PSUM") as psum_tp,
        ):
            # ---- constants ----
            ident = persist.tile([128, 128], BF16, tag="ident")
            make_identity(nc, ident[:])
            cb = persist.tile([128, 1], F32, tag="cb")
            nc.vector.memset(cb[:], C_MAGIC)

            if not g_is_ones:
                g_row = persist.tile([1, K], F32, tag="g_row")
                nc.sync.dma_start(g_row[:], g_d[:])
                g_b = persist.tile([128, K], F32, tag="g_b")
                nc.gpsimd.partition_broadcast(g_b[:], g_row[0:1, :])

            wTT = [
                persist.tile([128, KC, N // NH], BF16, tag=f"wTT{h}",
                             name=f"wTT{h}")
                for h in range(NH)
            ]
            whold = [
                persist.tile([128, K], F32, tag=f"whold{s}",
                             name=f"whold{s}")
                for s in range(S_HOLD)
            ]
            wpart = persist.tile([128, WS], F32, tag="wpart")
            wall = persist.tile([128, WS], F32, tag="wall")
            wsb = persist.tile([128, 1], F32, tag="wsb")
            invb = persist.tile([128, 1], F32, tag="invb")

            xqT_tiles = {}
            cs_tiles = {}
            xsc_tiles = {}

            # ================= emission helpers =================

            def emit_tp(src_bf16, dst_for_g, name):
                """Transpose [128, K] bf16 via identity matmuls, 4 chunks per
                PSUM bank; copy back g0 on DVE, g1 on ACT."""
                for g in range(KC // 4):
                    tp = psum_tp.tile([128, 512], F32, tag="tp",
                                      name=f"tp_{name}_{g}")
                    for jj in range(4):
                        j = g * 4 + jj
                        nc.tensor.matmul(
                            tp[:, jj * 128:(jj + 1) * 128],
                            lhsT=src_bf16[:, j * 128:(j + 1) * 128],
                            rhs=ident[:])
                    if g == 0:
                        nc.vector.tensor_copy(dst_for_g(g), tp[:])
                    else:
                        nc.scalar.copy(dst_for_g(g), tp[:])

            def emit_x_load(t, eng):
                xt = xt_pool.tile([128, K], F32, tag="xt", name=f"xt{t}")
                eng.dma_start(xt[:], x_d[t * 128:(t + 1) * 128, :])
                return xt

            def emit_x_quant(t, xt, late):
                """RMSNorm stats + int8 quant -> xq bf16; transpose emitted
                separately. late=True routes ssq/xq to gpsimd."""
                with nc.named_scope("x_quant"):
                    if g_is_ones:
                        xg = xt
                    else:
                        xg = xt_pool.tile([128, K], F32, tag="xg",
                                          name=f"xg{t}")
                        nc.vector.tensor_mul(xg[:], xt[:], g_b[:])

                    xsq = scr_pool.tile([128, K], BF16, tag="xsq",
                                        name=f"xsq{t}")
                    ssq = st_pool.tile([128, 1], F32, tag="ssq")
                    nc.vector.scalar_tensor_tensor(
                        out=xsq[:], in0=xt[:], scalar=1.0, in1=xt[:],
                        op0=ALU.mult, op1=ALU.mult, accum_out=ssq[:])
                    am = st_pool.tile([128, 1], F32, tag="am")
                    nc.vector.tensor_reduce(
                        am[:], xg[:], axis=mybir.AxisListType.X, op=ALU.max,
                        apply_absolute_value=True)

                    # rs = 1/sqrt(ms + eps) with one Newton step on sqrt
                    ms = st_pool.tile([128, 1], F32, tag="ms")
                    nc.vector.tensor_scalar(
                        out=ms[:], in0=ssq[:], scalar1=1.0 / K,
                        scalar2=NORM_EPS, op0=ALU.mult, op1=ALU.add)
                    s0 = st_pool.tile([128, 1], F32, tag="s0")
                    nc.scalar.sqrt(s0[:], ms[:])
                    r0 = st_pool.tile([128, 1], F32, tag="r0")
                    nc.vector.reciprocal(r0[:], s0[:])
                    t0 = st_pool.tile([128, 1], F32, tag="t0")
                    nc.vector.tensor_mul(t0[:], ms[:], r0[:])
                    t1 = st_pool.tile([128, 1], F32, tag="t1")
                    nc.vector.tensor_add(t1[:], t0[:], s0[:])
                    s1 = st_pool.tile([128, 1], F32, tag="s1")
                    nc.vector.tensor_scalar(
                        out=s1[:], in0=t1[:], scalar1=0.5,
                        scalar2=None, op0=ALU.mult)
                    rs = st_pool.tile([128, 1], F32, tag="rs")
                    nc.vector.reciprocal(rs[:], s1[:])

                    axr = st_pool.tile([128, 1], F32, tag="axr")
                    nc.vector.tensor_mul(axr[:], am[:], rs[:])
                    xsc = st_pool.tile([128, 1], F32, tag="xsc",
                                       name=f"xsc{t}")
                    nc.vector.tensor_scalar(
                        out=xsc[:], in0=axr[:], scalar1=1.0 / 127.0,
                        scalar2=None, op0=ALU.mult)
                    xsc_tiles[t] = xsc
                    sx = st_pool.tile([128, 1], F32, tag="sx")
                    nc.vector.tensor_scalar(
                        out=sx[:], in0=axr[:], scalar1=1.0 / 127.0,
                        scalar2=Q_EPS, op0=ALU.mult, op1=ALU.add)
                    dx = st_pool.tile([128, 1], F32, tag="dx")
                    nc.vector.reciprocal(dx[:], sx[:])
                    srow = st_pool.tile([128, 1], F32, tag="srow")
                    nc.vector.tensor_mul(srow[:], rs[:], dx[:])

                    # x_q = RNE(xg * srow) via +C (ACT) then -C
                    ux = ux_pool.tile([128, K], F32, tag="ux", name=f"ux{t}")
                    nc.scalar.activation(
                        ux[:], xg[:], AF.Identity,
                        bias=cb[:, 0:1], scale=srow[:, 0:1])
                    xq = xq_pool.tile([128, K], BF16, tag="xq", name=f"xq{t}")
                    eng_xq = nc.gpsimd if late else nc.vector
                    eng_xq.tensor_scalar(
                        out=xq[:], in0=ux[:], scalar1=C_MAGIC,
                        scalar2=None, op0=ALU.subtract)
                    return xq

            def emit_cs(t):
                cs = cs_pool.tile([128, 1], F32, tag="cs", name=f"cs{t}")
                nc.vector.tensor_mul(cs[:], xsc_tiles[t][:], wsb[:])
                cs_tiles[t] = cs

            def emit_x_tp(t, xq):
                xqT = xqT_pool.tile([128, KC, 128], BF16, tag="xqT",
                                    name=f"xqT{t}")
                emit_tp(xq, lambda g: xqT[:, g * 4:(g + 1) * 4, :], f"x{t}")
                xqT_tiles[t] = xqT

            def emit_w_load(s, eng):
                if s < S_HOLD:
                    dst = whold[s]
                else:
                    dst = w1_pool.tile([128, K], F32, tag="w1",
                                       name=f"w1_{s}")
                eng.dma_start(dst[:], w_d[s * 128:(s + 1) * 128, :])
                return dst

            def emit_w_abs(s, src):
                # DVE abs+accum (ACT queue must stay free for DMA triggers)
                wab = scr_pool.tile([128, K], BF16, tag="wab", name=f"wab{s}")
                nc.vector.scalar_tensor_tensor(
                    out=wab[:], in0=src[:], scalar=-1.0, in1=src[:],
                    op0=ALU.mult, op1=ALU.max,
                    accum_out=wpart[:, s:s + 1])

            def emit_w_reload(s, eng, pool):
                dst = pool.tile([128, K], F32, tag="w1" if pool is w1_pool
                                else "h1", name=f"wr{s}")
                eng.dma_start(dst[:], w_d[s * 128:(s + 1) * 128, :])
                return dst

            def emit_ternarize(s, src, cast_eng):
                """w_t^T strip: clip+RNE to {-1,0,1} then transpose."""
                with nc.named_scope("w_ternarize"):
                    u = uv_pool.tile([128, K], F32, tag="uv", name=f"wu{s}")
                    nc.vector.tensor_scalar(
                        out=u[:], in0=src[:], scalar1=invb[:, 0:1],
                        scalar2=1.0, op0=ALU.mult, op1=ALU.min)
                    v = uv_pool.tile([128, K], F32, tag="uv", name=f"wv{s}")
                    nc.vector.tensor_scalar(
                        out=v[:], in0=u[:], scalar1=-1.0,
                        scalar2=C_MAGIC, op0=ALU.max, op1=ALU.add)
                    wtn = wtn_pool.tile([128, K], BF16, tag="wtn",
                                        name=f"wtn{s}")
                    if cast_eng is nc.scalar:
                        nc.scalar.activation(wtn[:], v[:], AF.Copy,
                                             bias=-C_MAGIC)
                    else:
                        cast_eng.tensor_scalar(
                            out=wtn[:], in0=v[:], scalar1=C_MAGIC,
                            scalar2=None, op0=ALU.subtract)
                    h, hcol = s // (WS // NH), (s % (WS // NH)) * 128
                    emit_tp(wtn,
                            lambda g: wTT[h][:, g * 4:(g + 1) * 4,
                                             hcol:hcol + 128],
                            f"w{s}")

            def emit_mm(rt, h, gi):
                xqT = xqT_tiles[rt]
                with nc.named_scope("mm"):
                    pst = [
                        psum_mm.tile([128, 512], F32, tag="pmm",
                                     name=f"pmm_{rt}_{h}_{q}")
                        for q in range(4)
                    ]
                    for j in range(KC):
                        for q in range(4):
                            nc.tensor.matmul(
                                pst[q][:],
                                lhsT=xqT[:, j, :],
                                rhs=wTT[h][:, j, q * 512:(q + 1) * 512],
                                start=(j == 0), stop=(j == KC - 1))
                with nc.named_scope("out_scale"):
                    cs = cs_tiles[rt]
                    stg = stage_pool.tile([128, N // NH], F32, tag="stage",
                                          name=f"stg{rt}_{h}")
                    for q in range(4):
                        dst = stg[:, q * 512:(q + 1) * 512]
                        if q < 2:
                            nc.scalar.activation(
                                dst, pst[q][:], AF.Copy, scale=cs[:, 0:1])
                        else:
                            nc.vector.tensor_scalar(
                                out=dst, in0=pst[q][:], scalar1=cs[:, 0:1],
                                scalar2=None, op0=ALU.mult)
                    nc.scalar.dma_start(
                        out_d[rt * 128:(rt + 1) * 128,
                              h * 2048:(h + 1) * 2048],
                        stg[:])

            # ================= emission schedule =================

            # scalar ring first: all 32 w pass-1 strip loads (triggers run
            # at t=0; the w1 pool self-paces against the DVE abs chain)
            w_src = []
            for s in range(WS):
                w_src.append(emit_w_load(s, nc.scalar))

            # early x tiles 0,1 on sync ring + full quant + transpose
            for t in (0, 1):
                xt = emit_x_load(t, nc.sync)
                xq = emit_x_quant(t, xt, late=False)
                emit_x_tp(t, xq)

            # x2,3 loads early too (cheap, keeps sync ring busy)
            xt_early = {t: emit_x_load(t, nc.sync) for t in (2, 3)}

            # sync ring: re-loads of non-held burst strips (S_HOLD..15)
            burst_src = {s: w_src[s] for s in range(S_HOLD)}
            for s in range(S_HOLD, 16):
                burst_src[s] = emit_w_reload(s, nc.sync, w1_pool)

            # abs accumulation pass (DVE, DMA-paced)
            with nc.named_scope("w_abs_sum"):
                for s in range(WS):
                    emit_w_abs(s, w_src[s])

                # w_scale = mean|w|; inv = 1/(w_scale + eps)
                nc.gpsimd.partition_all_reduce(
                    wall[:], wpart[:], channels=128,
                    reduce_op=bass_isa.ReduceOp.add)
                wsumb = st_pool.tile([128, 1], F32, tag="wsumb")
                nc.vector.reduce_sum(wsumb[:], wall[:],
                                     axis=mybir.AxisListType.X)
                nc.vector.tensor_scalar(
                    out=wsb[:], in0=wsumb[:], scalar1=1.0 / (N * K),
                    scalar2=None, op0=ALU.mult)
                speps = st_pool.tile([128, 1], F32, tag="speps")
                nc.vector.tensor_scalar(
                    out=speps[:], in0=wsumb[:], scalar1=1.0 / (N * K),
                    scalar2=Q_EPS, op0=ALU.mult, op1=ALU.add)
                nc.vector.reciprocal(invb[:], speps[:])

            emit_cs(0)
            emit_cs(1)

            # h0 ternarize burst (ACT casts; PE transposes follow)
            for s in range(16):
                emit_ternarize(s, burst_src[s], nc.scalar)

            # h1 strip re-loads: first 3 on sync now, rest interleaved below
            h1_src = {}
            for s in range(16, 19):
                h1_src[s] = emit_w_reload(s, nc.sync, h1_pool)

            # x quant chains for tiles 2,3 (post-burst on DVE)
            for t in (2, 3):
                xq = emit_x_quant(t, xt_early[t], late=True)
                emit_cs(t)
                # transpose emitted in the mm phase (right before use)
                xt_early[t] = xq

            # ---- mm phase ----
            G = [(t, 0) for t in range(6)]
            tail0 = [(t, 0) for t in range(6, 16)]
            tail1 = [(t, 1) for t in range(10)]
            for a, b in zip(tail0, tail1):
                G.append(a)
                G.append(b)
            G += [(t, 1) for t in range(10, 16)]

            # h1 strips consumed at groups 0..4 (3,3,3,3,4); their loads are
            # interleaved two groups ahead
            h1_proc = {0: [16, 17, 18], 1: [19, 20, 21], 2: [22, 23, 24],
                       3: [25, 26, 27], 4: [28, 29, 30, 31]}
            h1_load = {0: [19, 20, 21], 1: [22, 23, 24], 2: [25, 26, 27],
                       3: [28, 29, 30, 31]}
            # x loads 4..15 trickle one per group; quant chains one per group
            xq_pending = dict(xt_early)

            for gi, (rt, h) in enumerate(G):
                # dma triggers (sync ring): next x tile, next h1 strips
                t_load = 4 + gi
                if t_load <= 15:
                    xt_early[t_load] = emit_x_load(t_load, nc.sync)
                for s in h1_load.get(gi, []):
                    h1_src[s] = emit_w_reload(s, nc.sync, h1_pool)

                # x quant chain for one pending tile
                t_q = 4 + gi
                if t_q <= 15:
                    xq = emit_x_quant(t_q, xt_early[t_q], late=True)
                    emit_cs(t_q)
                    xq_pending[t_q] = xq

                # transpose for this group's row tile (if not done yet)
                if h == 0 and rt >= 2:
                    emit_x_tp(rt, xq_pending.pop(rt))

                emit_mm(rt, h, gi)

                # h1 ternarize + transposes after the group's matmuls
                for s in h1_proc.get(gi, []):
                    emit_ternarize(s, h1_src[s], nc.scalar)

    nc.compile()
    return nc


def _ensure_ntff_hook():
    """Make trace=True work: bass_utils imports antenv.axon_hooks, which is
    not present in this image. Shim it and install the ctypes-based NTFF
    profiling hook against libaxon_pjrt.so (same recipe as trn_boot)."""
    import sys
    import types
    try:
        import antenv.axon_hooks  # noqa: F401
        return
    except ImportError:
        pass
    mod = types.ModuleType("antenv.axon_hooks")
    mod._hook = None
    mod.set_axon_ntff_profile_hook = lambda h: setattr(mod, "_hook", h)
    mod.get_axon_ntff_profile_hook = lambda: mod._hook
    sys.modules["antenv.axon_hooks"] = mod
    import antenv
    antenv.axon_hooks = mod
    try:
        from trn_agent_boot.trn_boot import _ntff_profile_via_ctypes
        hook = _ntff_profile_via_ctypes("/opt/axon/libaxon_pjrt.so")
        if hook is not None:
            mod._hook = hook
    except Exception as e:  # degrade to no-trace
        print(f"ntff hook install failed: {e}")
    # no S3 in this sandbox; keep artifacts local
    import concourse.bass_utils as bu
    bu.upload_artifacts = lambda tmpdir: f"local://{tmpdir}"


_NC_CACHE = {}


def kernel(x: np.ndarray, weight: np.ndarray, norm_weight: np.ndarray) -> np.ndarray:
    x = np.ascontiguousarray(x, dtype=np.float32)
    weight = np.ascontiguousarray(weight, dtype=np.float32)
    norm_weight = np.ascontiguousarray(norm_weight, dtype=np.float32)

    B, S, Kin = x.shape
    xf = x.reshape(-1, Kin)
    g_is_ones = bool(np.all(norm_weight == 1.0))

    if g_is_ones not in _NC_CACHE:
        _NC_CACHE[g_is_ones] = build_nc(g_is_ones)
    nc = _NC_CACHE[g_is_ones]

    in_maps = []
    for i in range(N_CORES):
        m = {"x": xf[i * R:(i + 1) * R], "w": weight}
        if not g_is_ones:
            m["g"] = norm_weight.reshape(1, Kin)
        in_maps.append(m)

    trace = bool(int(os.environ.get("BITLIN_TRACE", "0")))
    if trace:
        _ensure_ntff_hook()
    res = run_bass_kernel_spmd(
        nc, in_maps, core_ids=list(range(N_CORES)), trace=trace,
    )
    if trace:
        kernel.last_results = res
    out = np.concatenate([r["out"] for r in res.results], axis=0)
    return out.reshape(B, S, weight.shape[0]).astype(np.float32)


# revision 13
# speedup vs baseline: 1.3979x; 1.3979x over previous
"""BitLinear forward (RMSNorm -> int8 activation quant -> ternary weight quant
-> matmul -> rescale) on 8 Trainium2 NeuronCores.

Sharding: data-parallel over rows. x (4,4096,1024) flattens to (16384,1024);
each core gets 2048 rows and the full weight (4096,1024). w_scale=mean|w| is
computed locally per core from a single pass over the full weight (the
per-shard approximation fails the tolerance; a collective AllReduce costs
~55us of latency, while the single-pass local sum is DMA-bandwidth-bound at
~50us anyway and needs no collective).

v2 schedule (single kernel, emission order == per-engine FIFO order):
 - scalar HWDGE ring: 32 w-strip loads (pass 1), then all output stores.
 - sync HWDGE ring: x tiles + re-loads of the strips not held in SBUF.
 - DVE: |w| abs-accum per strip (keeps ACT free for DMA triggers), x quant
   chains, ternarize clip ops, half the PSUM evacuations.
 - ACT: sqrt + RNE-scale ops of x quant, h1 ternarize cast, half the PSUM
   evacuations.
 - GPSIMD: partition all-reduce for w_scale, h0 ternarize cast, x ssq/xq.
 - PE: identity-matmul transposes + the 1024 main matmuls; strict queue
   order chosen so the PE never head-of-line blocks on not-yet-ready work
   (that blocking caused ~110us of PE idle + HAM re-throttle in v1).

Math notes:
 - x_q are exact integers in [-128,127] and w_t in {-1,0,1}; both are exact in
   bf16, so a bf16 matmul with fp32 PSUM accumulation reproduces the fp32
   reference einsum bit-for-bit (|sums| < 2^24).
 - round-half-to-even is done in fp32 via the magic constant 1.5*2^23.
 - ternary quantize sign(ws)*(|ws|>0.5) == RNE(clip(ws,-1,1)) exactly.
 - transposes to [k, r]/[k, n] layouts are identity matmuls (out = a.T @ I),
   batched 4 chunks per PSUM bank with one wide copy back to SBUF.
"""

import os

import numpy as np

import concourse.bass as bass
import concourse.mybir as mybir
import concourse.tile as tile
from concourse import bacc
from concourse.bass_utils import run_bass_kernel_spmd
from concourse.masks import make_identity
from concourse import bass_isa

F32 = mybir.dt.float32
BF16 = mybir.dt.bfloat16
ALU = mybir.AluOpType
AF = mybir.ActivationFunctionType

N_CORES = 8
R_FULL, K, N = 16384, 1024, 4096
R = R_FULL // N_CORES          # 2048 rows per core
RT = R // 128                  # 16 row tiles per core
KC = K // 128                  # 8 k-chunks
WS = N // 128                  # 32 weight strips (of 128 out-features)
NH = 2                         # n halves (2048 each)
S_HOLD = 8                     # strips kept resident between pass1 and burst

C_MAGIC = 12582912.0           # 1.5 * 2^23: fp32 round-to-nearest-even trick
Q_EPS = 1e-5
NORM_EPS = 1e-6


def build_nc(g_is_ones: bool):
    nc = bacc.Bacc("TRN2", target_bir_lowering=False)

    x_d = nc.dram_tensor("x", [R, K], F32, kind="ExternalInput")
    w_d = nc.dram_tensor("w", [N, K], F32, kind="ExternalInput")
    if not g_is_ones:
        g_d = nc.dram_tensor("g", [1, K], F32, kind="ExternalInput")
    out_d = nc.dram_tensor("out", [R, N], F32, kind="ExternalOutput")

    with tile.TileContext(nc) as tc:
        with (
            tc.tile_pool(name="persist", bufs=1) as persist,
            tc.tile_pool(name="xt", bufs=2) as xt_pool,
            tc.tile_pool(name="scr", bufs=2) as scr_pool,       # bf16 scratch
            tc.tile_pool(name="st", bufs=2) as st_pool,         # [128,1] stats
            tc.tile_pool(name="ux", bufs=1) as ux_pool,
            tc.tile_pool(name="xqp", bufs=2) as xq_pool,
            tc.tile_pool(name="xqT", bufs=14) as xqT_pool,
            tc.tile_pool(name="w1", bufs=3) as w1_pool,         # w pass1 + reloads
            tc.tile_pool(name="h1p", bufs=3) as h1_pool,        # h1 reloads
            tc.tile_pool(name="uv", bufs=2) as uv_pool,
            tc.tile_pool(name="wtn", bufs=2) as wtn_pool,
            tc.tile_pool(name="stg", bufs=2) as stage_pool,
            tc.tile_pool(name="csp", bufs=16) as cs_pool,
            tc.tile_pool(name="pmm", bufs=6, space="PSUM") as psum_mm,
            tc.tile_pool(name="ptp", bufs=2, space="PSUM") as psum_tp,
        ):
            # ---- constants ----
            ident = persist.tile([128, 128], BF16, tag="ident")
            make_identity(nc, ident[:])
            cb = persist.tile([128, 1], F32, tag="cb")
            nc.vector.memset(cb[:], C_MAGIC)

            if not g_is_ones:
                g_row = persist.tile([1, K], F32, tag="g_row")
                nc.sync.dma_start(g_row[:], g_d[:])
                g_b = persist.tile([128, K], F32, tag="g_b")
                nc.gpsimd.partition_broadcast(g_b[:], g_row[0:1, :])

            wTT = [
                persist.tile([128, KC, N // NH], BF16, tag=f"wTT{h}",
                             name=f"wTT{h}")
                for h in range(NH)
            ]
            whold = [
                persist.tile([128, K], F32, tag=f"whold{s}",
                             name=f"whold{s}")
                for s in range(S_HOLD)
            ]
            wpart = persist.tile([128, WS], F32, tag="wpart")
            wall = persist.tile([128, WS], F32, tag="wall")
            wsb = persist.tile([128, 1], F32, tag="wsb")
            invb = persist.tile([128, 1], F32, tag="invb")

            xqT_tiles = {}
            cs_tiles = {}
            xsc_tiles = {}

            # ================= emission helpers =================

            def emit_tp(src_bf16, dst_for_g, name):
                """Transpose [128, K] bf16 via identity matmuls, 4 chunks per
                PSUM bank; copy back g0 on DVE, g1 on ACT."""
                for g in range(KC // 4):
                    tp = psum_tp.tile([128, 512], F32, tag="tp",
                                      name=f"tp_{name}_{g}")
                    for jj in range(4):
                        j = g * 4 + jj
                        nc.tensor.matmul(
                            tp[:, jj * 128:(jj + 1) * 128],
                            lhsT=src_bf16[:, j * 128:(j + 1) * 128],
                            rhs=ident[:])
                    if g == 0:
                        nc.vector.tensor_copy(dst_for_g(g), tp[:])
                    else:
                        nc.scalar.copy(dst_for_g(g), tp[:])

            def emit_x_load(t, eng):
                xt = xt_pool.tile([128, K], F32, tag="xt", name=f"xt{t}")
                eng.dma_start(xt[:], x_d[t * 128:(t + 1) * 128, :])
                return xt

            def emit_x_quant(t, xt, late):
                """RMSNorm stats + int8 quant -> xq bf16; transpose emitted
                separately. late=True routes ssq/xq to gpsimd."""
                with nc.named_scope("x_quant"):
                    if g_is_ones:
                        xg = xt
                    else:
                        xg = xt_pool.tile([128, K], F32, tag="xg",
                                          name=f"xg{t}")
                        nc.vector.tensor_mul(xg[:], xt[:], g_b[:])

                    xsq = scr_pool.tile([128, K], BF16, tag="xsq",
                                        name=f"xsq{t}")
                    ssq = st_pool.tile([128, 1], F32, tag="ssq")
                    nc.vector.scalar_tensor_tensor(
                        out=xsq[:], in0=xt[:], scalar=1.0, in1=xt[:],
                        op0=ALU.mult, op1=ALU.mult, accum_out=ssq[:])
                    am = st_pool.tile([128, 1], F32, tag="am")
                    nc.vector.tensor_reduce(
                        am[:], xg[:], axis=mybir.AxisListType.X, op=ALU.max,
                        apply_absolute_value=True)

                    # rs = 1/sqrt(ms + eps) with one Newton step on sqrt
                    ms = st_pool.tile([128, 1], F32, tag="ms")
                    nc.vector.tensor_scalar(
                        out=ms[:], in0=ssq[:], scalar1=1.0 / K,
                        scalar2=NORM_EPS, op0=ALU.mult, op1=ALU.add)
                    s0 = st_pool.tile([128, 1], F32, tag="s0")
                    nc.scalar.sqrt(s0[:], ms[:])
                    r0 = st_pool.tile([128, 1], F32, tag="r0")
                    nc.vector.reciprocal(r0[:], s0[:])
                    t0 = st_pool.tile([128, 1], F32, tag="t0")
                    nc.vector.tensor_mul(t0[:], ms[:], r0[:])
                    t1 = st_pool.tile([128, 1], F32, tag="t1")
                    nc.vector.tensor_add(t1[:], t0[:], s0[:])
                    s1 = st_pool.tile([128, 1], F32, tag="s1")
                    nc.vector.tensor_scalar(
                        out=s1[:], in0=t1[:], scalar1=0.5,
                        scalar2=None, op0=ALU.mult)
                    rs = st_pool.tile([128, 1], F32, tag="rs")
                    nc.vector.reciprocal(rs[:], s1[:])

                    axr = st_pool.tile([128, 1], F32, tag="axr")
                    nc.vector.tensor_mul(axr[:], am[:], rs[:])
                    xsc = st_pool.tile([128, 1], F32, tag="xsc",
                                       name=f"xsc{t}")
                    nc.vector.tensor_scalar(
                        out=xsc[:], in0=axr[:], scalar1=1.0 / 127.0,
                        scalar2=None, op0=ALU.mult)
                    xsc_tiles[t] = xsc
                    sx = st_pool.tile([128, 1], F32, tag="sx")
                    nc.vector.tensor_scalar(
                        out=sx[:], in0=axr[:], scalar1=1.0 / 127.0,
                        scalar2=Q_EPS, op0=ALU.mult, op1=ALU.add)
                    dx = st_pool.tile([128, 1], F32, tag="dx")
                    nc.vector.reciprocal(dx[:], sx[:])
                    srow = st_pool.tile([128, 1], F32, tag="srow")
                    nc.vector.tensor_mul(srow[:], rs[:], dx[:])

                    # x_q = RNE(xg * srow) via +C (ACT) then -C
                    ux = ux_pool.tile([128, K], F32, tag="ux", name=f"ux{t}")
                    nc.scalar.activation(
                        ux[:], xg[:], AF.Identity,
                        bias=cb[:, 0:1], scale=srow[:, 0:1])
                    xq = xq_pool.tile([128, K], BF16, tag="xq", name=f"xq{t}")
                    nc.vector.tensor_scalar(
                        out=xq[:], in0=ux[:], scalar1=C_MAGIC,
                        scalar2=None, op0=ALU.subtract)
                    return xq

            def emit_cs(t):
                cs = cs_pool.tile([128, 1], F32, tag="cs", name=f"cs{t}")
                nc.vector.tensor_mul(cs[:], xsc_tiles[t][:], wsb[:])
                cs_tiles[t] = cs

            def emit_x_tp(t, xq):
                xqT = xqT_pool.tile([128, KC, 128], BF16, tag="xqT",
                                    name=f"xqT{t}")
                emit_tp(xq, lambda g: xqT[:, g * 4:(g + 1) * 4, :], f"x{t}")
                xqT_tiles[t] = xqT

            def emit_w_load(s, eng):
                if s < S_HOLD:
                    dst = whold[s]
                else:
                    dst = w1_pool.tile([128, K], F32, tag="w1",
                                       name=f"w1_{s}")
                eng.dma_start(dst[:], w_d[s * 128:(s + 1) * 128, :])
                return dst

            def emit_w_abs(s, src):
                # DVE abs+accum (ACT queue must stay free for DMA triggers)
                wab = scr_pool.tile([128, K], BF16, tag="wab", name=f"wab{s}")
                nc.vector.scalar_tensor_tensor(
                    out=wab[:], in0=src[:], scalar=-1.0, in1=src[:],
                    op0=ALU.mult, op1=ALU.max,
                    accum_out=wpart[:, s:s + 1])

            def emit_w_reload(s, eng, pool):
                dst = pool.tile([128, K], F32, tag="w1" if pool is w1_pool
                                else "h1", name=f"wr{s}")
                eng.dma_start(dst[:], w_d[s * 128:(s + 1) * 128, :])
                return dst

            def emit_ternarize(s, src, cast_eng):
                """w_t^T strip: clip+RNE to {-1,0,1} then transpose."""
                with nc.named_scope("w_ternarize"):
                    u = uv_pool.tile([128, K], F32, tag="uv", name=f"wu{s}")
                    nc.vector.tensor_scalar(
                        out=u[:], in0=src[:], scalar1=invb[:, 0:1],
                        scalar2=1.0, op0=ALU.mult, op1=ALU.min)
                    v = uv_pool.tile([128, K], F32, tag="uv", name=f"wv{s}")
                    nc.vector.tensor_scalar(
                        out=v[:], in0=u[:], scalar1=-1.0,
                        scalar2=C_MAGIC, op0=ALU.max, op1=ALU.add)
                    wtn = wtn_pool.tile([128, K], BF16, tag="wtn",
                                        name=f"wtn{s}")
                    if cast_eng is nc.scalar:
                        nc.scalar.activation(wtn[:], v[:], AF.Copy,
                                             bias=-C_MAGIC)
                    else:
                        cast_eng.tensor_scalar(
                            out=wtn[:], in0=v[:], scalar1=C_MAGIC,
                            scalar2=None, op0=ALU.subtract)
                    h, hcol = s // (WS // NH), (s % (WS // NH)) * 128
                    emit_tp(wtn,
                            lambda g: wTT[h][:, g * 4:(g + 1) * 4,
                                             hcol:hcol + 128],
                            f"w{s}")

            def emit_mm(rt, h, gi):
                xqT = xqT_tiles[rt]
                with nc.named_scope("mm"):
                    pst = [
                        psum_mm.tile([128, 512], F32, tag="pmm",
                                     name=f"pmm_{rt}_{h}_{q}")
                        for q in range(4)
                    ]
                    for j in range(KC):
                        for q in range(4):
                            nc.tensor.matmul(
                                pst[q][:],
                                lhsT=xqT[:, j, :],
                                rhs=wTT[h][:, j, q * 512:(q + 1) * 512],
                                start=(j == 0), stop=(j == KC - 1))
                with nc.named_scope("out_scale"):
                    cs = cs_tiles[rt]
                    stg = stage_pool.tile([128, N // NH], F32, tag="stage",
                                          name=f"stg{rt}_{h}")
                    for q in range(4):
                        dst = stg[:, q * 512:(q + 1) * 512]
                        if q < 2:
                            nc.scalar.activation(
                                dst, pst[q][:], AF.Copy, scale=cs[:, 0:1])
                        else:
                            nc.vector.tensor_scalar(
                                out=dst, in0=pst[q][:], scalar1=cs[:, 0:1],
                                scalar2=None, op0=ALU.mult)
                    eng = nc.sync if gi % 2 == 0 else nc.scalar
                    eng.dma_start(
                        out_d[rt * 128:(rt + 1) * 128,
                              h * 2048:(h + 1) * 2048],
                        stg[:])

            # ================= emission schedule =================

            # x0,x1 first (one per ring), then w pass-1 split across both
            # rings; the w1 pool self-paces against the DVE abs chain
            xt_early = {}
            xt_early[0] = emit_x_load(0, nc.sync)
            xt_early[1] = emit_x_load(1, nc.scalar)
            w_src = []
            for s in range(WS):
                w_src.append(emit_w_load(s, nc.sync if s % 2 == 0
                                         else nc.scalar))
            xt_early[2] = emit_x_load(2, nc.sync)
            xt_early[3] = emit_x_load(3, nc.scalar)

            # re-loads of non-held burst strips (S_HOLD..15)
            burst_src = {s: w_src[s] for s in range(S_HOLD)}
            for s in range(S_HOLD, 16):
                burst_src[s] = emit_w_reload(
                    s, nc.sync if s % 2 == 0 else nc.scalar, w1_pool)

            # abs accumulation pass (DVE, DMA-paced)
            with nc.named_scope("w_abs_sum"):
                for s in range(WS):
                    emit_w_abs(s, w_src[s])

                # w_scale = mean|w|; inv = 1/(w_scale + eps)
                nc.gpsimd.partition_all_reduce(
                    wall[:], wpart[:], channels=128,
                    reduce_op=bass_isa.ReduceOp.add)
                wsumb = st_pool.tile([128, 1], F32, tag="wsumb")
                nc.vector.reduce_sum(wsumb[:], wall[:],
                                     axis=mybir.AxisListType.X)
                nc.vector.tensor_scalar(
                    out=wsb[:], in0=wsumb[:], scalar1=1.0 / (N * K),
                    scalar2=None, op0=ALU.mult)
                speps = st_pool.tile([128, 1], F32, tag="speps")
                nc.vector.tensor_scalar(
                    out=speps[:], in0=wsumb[:], scalar1=1.0 / (N * K),
                    scalar2=Q_EPS, op0=ALU.mult, op1=ALU.add)
                nc.vector.reciprocal(invb[:], speps[:])

            # x0,x1 quant + transpose (DVE work sits after the abs pass so
            # the w stream is never blocked; xqT0/1 only needed at mm start)
            for t in (0, 1):
                xq = emit_x_quant(t, xt_early[t], late=False)
                emit_cs(t)
                emit_x_tp(t, xq)

            # h0 ternarize burst (ACT casts; PE transposes follow)
            for s in range(16):
                emit_ternarize(s, burst_src[s], nc.scalar)

            # h1 strip re-loads: first 3 now, rest interleaved below
            h1_src = {}
            for s in range(16, 19):
                h1_src[s] = emit_w_reload(
                    s, nc.sync if s % 2 == 0 else nc.scalar, h1_pool)

            # x quant chains for tiles 2,3 (post-burst on DVE)
            for t in (2, 3):
                xq = emit_x_quant(t, xt_early[t], late=True)
                emit_cs(t)
                # transpose emitted in the mm phase (right before use)
                xt_early[t] = xq

            # ---- mm phase ----
            G = [(t, 0) for t in range(6)]
            tail0 = [(t, 0) for t in range(6, 16)]
            tail1 = [(t, 1) for t in range(10)]
            for a, b in zip(tail0, tail1):
                G.append(a)
                G.append(b)
            G += [(t, 1) for t in range(10, 16)]

            # h1 strips consumed at groups 0..4 (3,3,3,3,4); their loads are
            # interleaved two groups ahead
            h1_proc = {0: [16, 17, 18], 1: [19, 20, 21], 2: [22, 23, 24],
                       3: [25, 26, 27], 4: [28, 29, 30, 31]}
            h1_load = {0: [19, 20, 21], 1: [22, 23, 24], 2: [25, 26, 27],
                       3: [28, 29, 30, 31]}
            # x loads 4..15 trickle one per group; quant chains one per group
            xq_pending = dict(xt_early)

            for gi, (rt, h) in enumerate(G):
                # dma triggers (sync ring): next x tile, next h1 strips
                t_load = 4 + gi
                if t_load <= 15:
                    xt_early[t_load] = emit_x_load(
                        t_load, nc.sync if t_load % 2 == 0 else nc.scalar)
                for s in h1_load.get(gi, []):
                    h1_src[s] = emit_w_reload(
                        s, nc.sync if s % 2 == 0 else nc.scalar, h1_pool)

                # x quant chain for one pending tile
                t_q = 4 + gi
                if t_q <= 15:
                    xq = emit_x_quant(t_q, xt_early[t_q], late=True)
                    emit_cs(t_q)
                    xq_pending[t_q] = xq

                # transpose for this group's row tile (if not done yet)
                if h == 0 and rt >= 2:
                    emit_x_tp(rt, xq_pending.pop(rt))

                emit_mm(rt, h, gi)

                # h1 ternarize + transposes after the group's matmuls
                for s in h1_proc.get(gi, []):
                    emit_ternarize(s, h1_src[s], nc.scalar)

    nc.compile()
    return nc


def _ensure_ntff_hook():
    """Make trace=True work: bass_utils imports antenv.axon_hooks, which is
    not present in this image. Shim it and install the ctypes-based NTFF
    profiling hook against libaxon_pjrt.so (same recipe as trn_boot)."""
    import sys
    import types
    try:
        import antenv.axon_hooks  # noqa: F401
        return
    except ImportError:
        pass
    mod = types.ModuleType("antenv.axon_hooks")
    mod._hook = None
    mod.set_axon_ntff_profile_hook = lambda h: setattr(mod, "_hook", h)
    mod.get_axon_ntff_profile_hook = lambda: mod._hook
    sys.modules["antenv.axon_hooks"] = mod
    import antenv
    antenv.axon_hooks = mod
    try:
        from trn_agent_boot.trn_boot import _ntff_profile_via_ctypes
        hook = _ntff_profile_via_ctypes("/opt/axon/libaxon_pjrt.so")
        if hook is not None:
            mod._hook = hook
    except Exception as e:  # degrade to no-trace
        print(f"ntff hook install failed: {e}")
    # no S3 in this sandbox; keep artifacts local
    import concourse.bass_utils as bu
    bu.upload_artifacts = lambda tmpdir: f"local://{tmpdir}"


_NC_CACHE = {}


def kernel(x: np.ndarray, weight: np.ndarray, norm_weight: np.ndarray) -> np.ndarray:
    x = np.ascontiguousarray(x, dtype=np.float32)
    weight = np.ascontiguousarray(weight, dtype=np.float32)
    norm_weight = np.ascontiguousarray(norm_weight, dtype=np.float32)

    B, S, Kin = x.shape
    xf = x.reshape(-1, Kin)
    g_is_ones = bool(np.all(norm_weight == 1.0))

    if g_is_ones not in _NC_CACHE:
        _NC_CACHE[g_is_ones] = build_nc(g_is_ones)
    nc = _NC_CACHE[g_is_ones]

    in_maps = []
    for i in range(N_CORES):
        m = {"x": xf[i * R:(i + 1) * R], "w": weight}
        if not g_is_ones:
            m["g"] = norm_weight.reshape(1, Kin)
        in_maps.append(m)

    trace = bool(int(os.environ.get("BITLIN_TRACE", "0")))
    if trace:
        _ensure_ntff_hook()
    res = run_bass_kernel_spmd(
        nc, in_maps, core_ids=list(range(N_CORES)), trace=trace,
    )
    if trace:
        kernel.last_results = res
    out = np.concatenate([r["out"] for r in res.results], axis=0)
    return out.reshape(B, S, weight.shape[0]).astype(np.float32)


# revision 22
# speedup vs baseline: 1.4256x; 1.0198x over previous
"""BitLinear forward (RMSNorm -> int8 activation quant -> ternary weight quant
-> matmul -> rescale) on 8 Trainium2 NeuronCores.

Sharding: data-parallel over rows. x (4,4096,1024) flattens to (16384,1024);
each core gets 2048 rows and the full weight (4096,1024). w_scale=mean|w| is
computed locally per core from a single pass over the full weight (the
per-shard approximation fails the tolerance; a collective AllReduce costs
~55us of latency, while the single-pass local sum is DMA-bandwidth-bound at
~50us anyway and needs no collective).

v2 schedule (single kernel, emission order == per-engine FIFO order):
 - scalar HWDGE ring: 32 w-strip loads (pass 1), then all output stores.
 - sync HWDGE ring: x tiles + re-loads of the strips not held in SBUF.
 - DVE: |w| abs-accum per strip (keeps ACT free for DMA triggers), x quant
   chains, ternarize clip ops, half the PSUM evacuations.
 - ACT: sqrt + RNE-scale ops of x quant, h1 ternarize cast, half the PSUM
   evacuations.
 - GPSIMD: partition all-reduce for w_scale, h0 ternarize cast, x ssq/xq.
 - PE: identity-matmul transposes + the 1024 main matmuls; strict queue
   order chosen so the PE never head-of-line blocks on not-yet-ready work
   (that blocking caused ~110us of PE idle + HAM re-throttle in v1).

Math notes:
 - x_q are exact integers in [-128,127] and w_t in {-1,0,1}; both are exact in
   bf16, so a bf16 matmul with fp32 PSUM accumulation reproduces the fp32
   reference einsum bit-for-bit (|sums| < 2^24).
 - round-half-to-even is done in fp32 via the magic constant 1.5*2^23.
 - ternary quantize sign(ws)*(|ws|>0.5) == RNE(clip(ws,-1,1)) exactly.
 - transposes to [k, r]/[k, n] layouts are identity matmuls (out = a.T @ I),
   batched 4 chunks per PSUM bank with one wide copy back to SBUF.
"""

import os

import numpy as np

import concourse.bass as bass
import concourse.mybir as mybir
import concourse.tile as tile
from concourse import bacc
from concourse.bass_utils import run_bass_kernel_spmd
from concourse.masks import make_identity
from concourse import bass_isa

F32 = mybir.dt.float32
BF16 = mybir.dt.bfloat16
ALU = mybir.AluOpType
AF = mybir.ActivationFunctionType

N_CORES = 8
R_FULL, K, N = 16384, 1024, 4096
R = R_FULL // N_CORES          # 2048 rows per core
RT = R // 128                  # 16 row tiles per core
KC = K // 128                  # 8 k-chunks
WS = N // 128                  # 32 weight strips (of 128 out-features)
NH = 2                         # n halves (2048 each)
S_HOLD = 8                     # strips kept resident between pass1 and burst

C_MAGIC = 12582912.0           # 1.5 * 2^23: fp32 round-to-nearest-even trick
Q_EPS = 1e-5
NORM_EPS = 1e-6


def build_nc(g_is_ones: bool):
    nc = bacc.Bacc("TRN2", target_bir_lowering=False)

    x_d = nc.dram_tensor("x", [R, K], F32, kind="ExternalInput")
    # w viewed as [16 batches, 2 strips, 128 rows, K] (row-major identical)
    w_d = nc.dram_tensor("w", [WS // 2, 2, 128, K], F32,
                         kind="ExternalInput")
    if not g_is_ones:
        g_d = nc.dram_tensor("g", [1, K], F32, kind="ExternalInput")
    out_d = nc.dram_tensor("out", [R, N], F32, kind="ExternalOutput")

    with tile.TileContext(nc) as tc:
        with (
            tc.tile_pool(name="persist", bufs=1) as persist,
            tc.tile_pool(name="xt", bufs=2) as xt_pool,
            tc.tile_pool(name="scr", bufs=2) as scr_pool,       # bf16 scratch
            tc.tile_pool(name="st", bufs=2) as st_pool,         # [128,1] stats
            tc.tile_pool(name="ux", bufs=1) as ux_pool,
            tc.tile_pool(name="xqp", bufs=2) as xq_pool,
            tc.tile_pool(name="xqT", bufs=8) as xqT_pool,
            tc.tile_pool(name="w1b", bufs=2) as w1b_pool,       # w pass1 batches
            tc.tile_pool(name="rp", bufs=2) as rp_pool,         # burst reloads
            tc.tile_pool(name="h1p", bufs=3) as h1_pool,        # h1 reloads
            tc.tile_pool(name="uv", bufs=2) as uv_pool,
            tc.tile_pool(name="wtn", bufs=2) as wtn_pool,
            tc.tile_pool(name="stg", bufs=2) as stage_pool,
            tc.tile_pool(name="csp", bufs=16) as cs_pool,
            tc.tile_pool(name="pmm", bufs=6, space="PSUM") as psum_mm,
            tc.tile_pool(name="ptp", bufs=2, space="PSUM") as psum_tp,
        ):
            # ---- constants ----
            ident = persist.tile([128, 128], BF16, tag="ident")
            make_identity(nc, ident[:])
            cb = persist.tile([128, 1], F32, tag="cb")
            nc.vector.memset(cb[:], C_MAGIC)

            if not g_is_ones:
                g_row = persist.tile([1, K], F32, tag="g_row")
                nc.sync.dma_start(g_row[:], g_d[:])
                g_b = persist.tile([128, K], F32, tag="g_b")
                nc.gpsimd.partition_broadcast(g_b[:], g_row[0:1, :])

            wTT = [
                persist.tile([128, KC, N // NH], BF16, tag=f"wTT{h}",
                             name=f"wTT{h}")
                for h in range(NH)
            ]
            whold = [
                persist.tile([128, 2, K], F32, tag=f"whold{b}",
                             name=f"whold{b}")
                for b in range(S_HOLD // 2)
            ]
            wpart = persist.tile([128, WS // 2], F32, tag="wpart")
            wall = persist.tile([128, WS // 2], F32, tag="wall")
            wsb = persist.tile([128, 1], F32, tag="wsb")
            invb = persist.tile([128, 1], F32, tag="invb")

            xqT_tiles = {}
            cs_tiles = {}
            xsc_tiles = {}

            # ================= emission helpers =================

            def emit_tp(src_bf16, dst_for_g, name):
                """Transpose [128, K] bf16 via identity matmuls, 4 chunks per
                PSUM bank; copy back g0 on DVE, g1 on ACT."""
                for g in range(KC // 4):
                    tp = psum_tp.tile([128, 512], F32, tag="tp",
                                      name=f"tp_{name}_{g}")
                    for jj in range(4):
                        j = g * 4 + jj
                        nc.tensor.matmul(
                            tp[:, jj * 128:(jj + 1) * 128],
                            lhsT=src_bf16[:, j * 128:(j + 1) * 128],
                            rhs=ident[:])
                    if g == 0:
                        nc.vector.tensor_copy(dst_for_g(g), tp[:])
                    else:
                        nc.scalar.copy(dst_for_g(g), tp[:])

            def emit_x_load(t, eng):
                xt = xt_pool.tile([128, K], F32, tag="xt", name=f"xt{t}")
                eng.dma_start(xt[:], x_d[t * 128:(t + 1) * 128, :])
                return xt

            def emit_x_quant(t, xt, late):
                """RMSNorm stats + int8 quant -> xq bf16; transpose emitted
                separately. late=True routes ssq/xq to gpsimd."""
                with nc.named_scope("x_quant"):
                    if g_is_ones:
                        xg = xt
                    else:
                        xg = xt_pool.tile([128, K], F32, tag="xg",
                                          name=f"xg{t}")
                        nc.vector.tensor_mul(xg[:], xt[:], g_b[:])

                    xsq = scr_pool.tile([128, K], BF16, tag="xsq",
                                        name=f"xsq{t}")
                    ssq = st_pool.tile([128, 1], F32, tag="ssq")
                    nc.vector.scalar_tensor_tensor(
                        out=xsq[:], in0=xt[:], scalar=1.0, in1=xt[:],
                        op0=ALU.mult, op1=ALU.mult, accum_out=ssq[:])
                    am = st_pool.tile([128, 1], F32, tag="am")
                    nc.vector.tensor_reduce(
                        am[:], xg[:], axis=mybir.AxisListType.X, op=ALU.max,
                        apply_absolute_value=True)

                    # rs = 1/sqrt(ms + eps) with one Newton step on sqrt
                    ms = st_pool.tile([128, 1], F32, tag="ms")
                    nc.vector.tensor_scalar(
                        out=ms[:], in0=ssq[:], scalar1=1.0 / K,
                        scalar2=NORM_EPS, op0=ALU.mult, op1=ALU.add)
                    s0 = st_pool.tile([128, 1], F32, tag="s0")
                    nc.scalar.sqrt(s0[:], ms[:])
                    r0 = st_pool.tile([128, 1], F32, tag="r0")
                    nc.vector.reciprocal(r0[:], s0[:])
                    t0 = st_pool.tile([128, 1], F32, tag="t0")
                    nc.vector.tensor_mul(t0[:], ms[:], r0[:])
                    t1 = st_pool.tile([128, 1], F32, tag="t1")
                    nc.vector.tensor_add(t1[:], t0[:], s0[:])
                    s1 = st_pool.tile([128, 1], F32, tag="s1")
                    nc.vector.tensor_scalar(
                        out=s1[:], in0=t1[:], scalar1=0.5,
                        scalar2=None, op0=ALU.mult)
                    rs = st_pool.tile([128, 1], F32, tag="rs")
                    nc.vector.reciprocal(rs[:], s1[:])

                    axr = st_pool.tile([128, 1], F32, tag="axr")
                    nc.vector.tensor_mul(axr[:], am[:], rs[:])
                    xsc = st_pool.tile([128, 1], F32, tag="xsc",
                                       name=f"xsc{t}")
                    nc.vector.tensor_scalar(
                        out=xsc[:], in0=axr[:], scalar1=1.0 / 127.0,
                        scalar2=None, op0=ALU.mult)
                    xsc_tiles[t] = xsc
                    sx = st_pool.tile([128, 1], F32, tag="sx")
                    nc.vector.tensor_scalar(
                        out=sx[:], in0=axr[:], scalar1=1.0 / 127.0,
                        scalar2=Q_EPS, op0=ALU.mult, op1=ALU.add)
                    dx = st_pool.tile([128, 1], F32, tag="dx")
                    nc.vector.reciprocal(dx[:], sx[:])
                    srow = st_pool.tile([128, 1], F32, tag="srow")
                    nc.vector.tensor_mul(srow[:], rs[:], dx[:])

                    # x_q = RNE(xg * srow) via +C (ACT) then -C
                    ux = ux_pool.tile([128, K], F32, tag="ux", name=f"ux{t}")
                    nc.scalar.activation(
                        ux[:], xg[:], AF.Identity,
                        bias=cb[:, 0:1], scale=srow[:, 0:1])
                    xq = xq_pool.tile([128, K], BF16, tag="xq", name=f"xq{t}")
                    nc.vector.tensor_scalar(
                        out=xq[:], in0=ux[:], scalar1=C_MAGIC,
                        scalar2=None, op0=ALU.subtract)
                    return xq

            def emit_cs(t):
                cs = cs_pool.tile([128, 1], F32, tag="cs", name=f"cs{t}")
                nc.vector.tensor_mul(cs[:], xsc_tiles[t][:], wsb[:])
                cs_tiles[t] = cs

            def emit_x_tp(t, xq):
                xqT = xqT_pool.tile([128, KC, 128], BF16, tag="xqT",
                                    name=f"xqT{t}")
                emit_tp(xq, lambda g: xqT[:, g * 4:(g + 1) * 4, :], f"x{t}")
                xqT_tiles[t] = xqT

            def emit_w_load(b, eng):
                """Load batch b (2 strips) partition-major in one DMA."""
                if b < S_HOLD // 2:
                    dst = whold[b]
                else:
                    dst = w1b_pool.tile([128, 2, K], F32, tag="w1b",
                                        name=f"w1b_{b}")
                eng.dma_start(dst[:], w_d[b].transpose([1, 0, 2]))
                return dst

            def emit_w_abs(b, src):
                # DVE abs+accum (ACT queue must stay free for DMA triggers)
                wab = scr_pool.tile([128, 2, K], BF16, tag="wab",
                                    name=f"wab{b}")
                nc.vector.scalar_tensor_tensor(
                    out=wab[:], in0=src[:], scalar=-1.0, in1=src[:],
                    op0=ALU.mult, op1=ALU.max,
                    accum_out=wpart[:, b:b + 1])

            def emit_w_reload(s, eng, pool):
                dst = pool.tile([128, K], F32, tag="rp" if pool is rp_pool
                                else "h1", name=f"wr{s}")
                eng.dma_start(dst[:], w_d[s // 2, s % 2])
                return dst

            def emit_ternarize(s, src, cast_eng):
                """w_t^T strip: clip+RNE to {-1,0,1} then transpose."""
                with nc.named_scope("w_ternarize"):
                    u = uv_pool.tile([128, K], F32, tag="uv", name=f"wu{s}")
                    nc.vector.tensor_scalar(
                        out=u[:], in0=src, scalar1=invb[:, 0:1],
                        scalar2=1.0, op0=ALU.mult, op1=ALU.min)
                    v = uv_pool.tile([128, K], F32, tag="uv", name=f"wv{s}")
                    nc.vector.tensor_scalar(
                        out=v[:], in0=u[:], scalar1=-1.0,
                        scalar2=C_MAGIC, op0=ALU.max, op1=ALU.add)
                    wtn = wtn_pool.tile([128, K], BF16, tag="wtn",
                                        name=f"wtn{s}")
                    if cast_eng is nc.scalar:
                        nc.scalar.activation(wtn[:], v[:], AF.Copy,
                                             bias=-C_MAGIC)
                    else:
                        cast_eng.tensor_scalar(
                            out=wtn[:], in0=v[:], scalar1=C_MAGIC,
                            scalar2=None, op0=ALU.subtract)
                    h, hcol = s // (WS // NH), (s % (WS // NH)) * 128
                    emit_tp(wtn,
                            lambda g: wTT[h][:, g * 4:(g + 1) * 4,
                                             hcol:hcol + 128],
                            f"w{s}")

            def emit_mm(rt, h, gi):
                xqT = xqT_tiles[rt]
                with nc.named_scope("mm"):
                    pst = [
                        psum_mm.tile([128, 512], F32, tag="pmm",
                                     name=f"pmm_{rt}_{h}_{q}")
                        for q in range(4)
                    ]
                    for j in range(KC):
                        for q in range(4):
                            nc.tensor.matmul(
                                pst[q][:],
                                lhsT=xqT[:, j, :],
                                rhs=wTT[h][:, j, q * 512:(q + 1) * 512],
                                start=(j == 0), stop=(j == KC - 1))
                with nc.named_scope("out_scale"):
                    cs = cs_tiles[rt]
                    stg = stage_pool.tile([128, N // NH], F32, tag="stage",
                                          name=f"stg{rt}_{h}")
                    for q in range(4):
                        dst = stg[:, q * 512:(q + 1) * 512]
                        if q < 2:
                            nc.scalar.activation(
                                dst, pst[q][:], AF.Copy, scale=cs[:, 0:1])
                        else:
                            nc.vector.tensor_scalar(
                                out=dst, in0=pst[q][:], scalar1=cs[:, 0:1],
                                scalar2=None, op0=ALU.mult)
                    eng = nc.sync if gi % 2 == 0 else nc.scalar
                    eng.dma_start(
                        out_d[rt * 128:(rt + 1) * 128,
                              h * 2048:(h + 1) * 2048],
                        stg[:])

            # ================= emission schedule =================

            # x0,x1 first (one per ring), then w pass-1 split across both
            # rings; the w1 pool self-paces against the DVE abs chain
            xt_early = {}
            xt_early[0] = emit_x_load(0, nc.sync)
            xt_early[1] = emit_x_load(1, nc.scalar)
            w_src = []
            for b in range(WS // 2):
                w_src.append(emit_w_load(b, nc.sync if b % 2 == 0
                                         else nc.scalar))
            xt_early[2] = emit_x_load(2, nc.sync)
            xt_early[3] = emit_x_load(3, nc.scalar)

            # re-loads of non-held burst strips (S_HOLD..15)
            burst_src = {s: whold[s // 2][:, s % 2, :]
                         for s in range(S_HOLD)}
            for s in range(S_HOLD, 16):
                burst_src[s] = emit_w_reload(
                    s, nc.sync if s % 2 == 0 else nc.scalar, rp_pool)[:]

            # abs accumulation pass (DVE, DMA-paced)
            with nc.named_scope("w_abs_sum"):
                for b in range(WS // 2):
                    emit_w_abs(b, w_src[b])

                # w_scale = mean|w|; inv = 1/(w_scale + eps)
                nc.gpsimd.partition_all_reduce(
                    wall[:], wpart[:], channels=128,
                    reduce_op=bass_isa.ReduceOp.add)
                wsumb = st_pool.tile([128, 1], F32, tag="wsumb")
                nc.vector.reduce_sum(wsumb[:], wall[:],
                                     axis=mybir.AxisListType.X)
                nc.vector.tensor_scalar(
                    out=wsb[:], in0=wsumb[:], scalar1=1.0 / (N * K),
                    scalar2=None, op0=ALU.mult)
                speps = st_pool.tile([128, 1], F32, tag="speps")
                nc.vector.tensor_scalar(
                    out=speps[:], in0=wsumb[:], scalar1=1.0 / (N * K),
                    scalar2=Q_EPS, op0=ALU.mult, op1=ALU.add)
                nc.vector.reciprocal(invb[:], speps[:])

            # x0,x1 quant + transpose (DVE work sits after the abs pass so
            # the w stream is never blocked; xqT0/1 only needed at mm start)
            for t in (0, 1):
                xq = emit_x_quant(t, xt_early[t], late=False)
                emit_cs(t)
                emit_x_tp(t, xq)

            # h0 ternarize burst (ACT casts; PE transposes follow)
            for s in range(16):
                emit_ternarize(s, burst_src[s], nc.scalar)

            # h1 strip re-loads: first 3 now, rest interleaved below
            h1_src = {}
            for s in range(16, 19):
                h1_src[s] = emit_w_reload(
                    s, nc.sync if s % 2 == 0 else nc.scalar, h1_pool)[:]

            # x quant chains for tiles 2,3 (post-burst on DVE)
            for t in (2, 3):
                xq = emit_x_quant(t, xt_early[t], late=True)
                emit_cs(t)
                # transpose emitted in the mm phase (right before use)
                xt_early[t] = xq

            # ---- mm phase ----
            G = [(t, 0) for t in range(6)]
            tail0 = [(t, 0) for t in range(6, 16)]
            tail1 = [(t, 1) for t in range(10)]
            for a, b in zip(tail0, tail1):
                G.append(a)
                G.append(b)
            G += [(t, 1) for t in range(10, 16)]

            # h1 strips consumed at groups 0..4 (3,3,3,3,4); their loads are
            # interleaved two groups ahead
            h1_proc = {0: [16, 17, 18], 1: [19, 20, 21], 2: [22, 23, 24],
                       3: [25, 26, 27], 4: [28, 29, 30, 31]}
            h1_load = {0: [19, 20, 21], 1: [22, 23, 24], 2: [25, 26, 27],
                       3: [28, 29, 30, 31]}
            # x loads 4..15 trickle one per group; quant chains one per group
            xq_pending = dict(xt_early)

            for gi, (rt, h) in enumerate(G):
                # dma triggers (sync ring): next x tile, next h1 strips
                t_load = 4 + gi
                if t_load <= 15:
                    xt_early[t_load] = emit_x_load(
                        t_load, nc.sync if t_load % 2 == 0 else nc.scalar)
                for s in h1_load.get(gi, []):
                    h1_src[s] = emit_w_reload(
                        s, nc.sync if s % 2 == 0 else nc.scalar, h1_pool)[:]

                # x quant chain for one pending tile
                t_q = 4 + gi
                if t_q <= 15:
                    xq = emit_x_quant(t_q, xt_early[t_q], late=True)
                    emit_cs(t_q)
                    xq_pending[t_q] = xq

                # transpose for this group's row tile (if not done yet)
                if h == 0 and rt >= 2:
                    emit_x_tp(rt, xq_pending.pop(rt))

                emit_mm(rt, h, gi)

                # h1 ternarize + transposes after the group's matmuls
                for s in h1_proc.get(gi, []):
                    emit_ternarize(s, h1_src[s], nc.scalar)

    nc.compile()
    return nc


def _ensure_ntff_hook():
    """Make trace=True work: bass_utils imports antenv.axon_hooks, which is
    not present in this image. Shim it and install the ctypes-based NTFF
    profiling hook against libaxon_pjrt.so (same recipe as trn_boot)."""
    import sys
    import types
    try:
        import antenv.axon_hooks  # noqa: F401
        return
    except ImportError:
        pass
    mod = types.ModuleType("antenv.axon_hooks")
    mod._hook = None
    mod.set_axon_ntff_profile_hook = lambda h: setattr(mod, "_hook", h)
    mod.get_axon_ntff_profile_hook = lambda: mod._hook
    sys.modules["antenv.axon_hooks"] = mod
    import antenv
    antenv.axon_hooks = mod
    try:
        from trn_agent_boot.trn_boot import _ntff_profile_via_ctypes
        hook = _ntff_profile_via_ctypes("/opt/axon/libaxon_pjrt.so")
        if hook is not None:
            mod._hook = hook
    except Exception as e:  # degrade to no-trace
        print(f"ntff hook install failed: {e}")
    # no S3 in this sandbox; keep artifacts local
    import concourse.bass_utils as bu
    bu.upload_artifacts = lambda tmpdir: f"local://{tmpdir}"


_NC_CACHE = {}


def kernel(x: np.ndarray, weight: np.ndarray, norm_weight: np.ndarray) -> np.ndarray:
    x = np.ascontiguousarray(x, dtype=np.float32)
    weight = np.ascontiguousarray(weight, dtype=np.float32)
    norm_weight = np.ascontiguousarray(norm_weight, dtype=np.float32)

    B, S, Kin = x.shape
    xf = x.reshape(-1, Kin)
    g_is_ones = bool(np.all(norm_weight == 1.0))

    if g_is_ones not in _NC_CACHE:
        _NC_CACHE[g_is_ones] = build_nc(g_is_ones)
    nc = _NC_CACHE[g_is_ones]

    wq = weight.reshape(WS // 2, 2, 128, Kin)
    in_maps = []
    for i in range(N_CORES):
        m = {"x": xf[i * R:(i + 1) * R], "w": wq}
        if not g_is_ones:
            m["g"] = norm_weight.reshape(1, Kin)
        in_maps.append(m)

    trace = bool(int(os.environ.get("BITLIN_TRACE", "0")))
    if trace:
        _ensure_ntff_hook()
    res = run_bass_kernel_spmd(
        nc, in_maps, core_ids=list(range(N_CORES)), trace=trace,
    )
    if trace:
        kernel.last_results = res
    out = np.concatenate([r["out"] for r in res.results], axis=0)
    return out.reshape(B, S, weight.shape[0]).astype(np.float32)
